# revision 10
# baseline (speedup 1.0000x reference)
"""CrossModalAttention Trainium2 kernel.

Math: with seq_len=1 on both query and key/value sides, softmax over the
single key is exactly 1.0, so MHA(q_in, kv_in) == (kv_in @ Wv.T + bv) @ out_w.T + out_b.
Folding the two projections on the host (in float64):
    W = out_w @ Wv          c = bv @ out_w.T + out_b
gives   out_m = LayerNorm(kv @ W.T + c + residual) * g + b.

Device work per modality: one [B,1024]x[1024,1024] matmul + residual add +
LayerNorm.  Sharding: pure data parallel over the batch dim, 8 cores.

Performance scheme (vs the fp32r baseline):
  * contraction split: first J of 8 k-chunks run as fp8(e4m3) DoubleRow
    matmuls (2 k-chunks per pass), the rest as bf16 matmuls.  J trades
    accuracy (fp8 quantization) for PE time; measured rel-err at J=6 is
    ~1.8e-2 vs the 2e-2 gate.
  * both modalities' lhsT are pre-transposed on the host (no PE
    transposes / PSUM->SBUF casts on device).
  * residual add runs on the Pool engine (gpsimd), LN stats on DVE,
    LN apply on the scalar engine; c (projection bias) is folded into
    the residual on the host.
  * all feature/weight traffic is bf16/fp8; outputs are written bf16
    and upcast on the host.
"""

import numpy as np
import ml_dtypes

P = 128          # partitions
D = 1024         # hidden dim
KO = D // P      # 8 contraction chunks
N_CORES = 8
B_FULL = 16384
B_CORE = B_FULL // N_CORES   # 2048
RT = B_CORE // P             # 16 row tiles per core
LN_EPS = 1e-5

J = 6            # fp8 k-chunks (DoubleRow pairs = J//2); 8-J chunks stay bf16
ADD_ENGINE = "pe"   # residual add: "pool" | "dve" | "pe"

NP_F8 = ml_dtypes.float8_e4m3
NP_BF16 = ml_dtypes.bfloat16

_PROGRAM_CACHE = {}


def _build_program(flags):
    """flags = (gb1, gb2): whether LN gamma/beta are non-trivial."""
    import concourse.bass as bass
    import concourse.bacc as bacc
    import concourse.tile as tile
    from concourse import mybir
    from concourse.masks import make_identity
    from concourse._compat import get_trn_type

    gb1, gb2 = flags
    f32 = mybir.dt.float32
    f8 = mybir.dt.float8e4
    bf16 = mybir.dt.bfloat16
    DR = mybir.MatmulPerfMode.DoubleRow
    JB = KO - J       # bf16 chunks

    nc = bacc.Bacc(get_trn_type() or "TRN2", target_bir_lowering=False,
                   debug=False, num_devices=N_CORES)

    # residuals (c folded in on host), [row, n] layout
    res_d = {1: nc.dram_tensor("res1", (B_CORE, D), bf16, kind="ExternalInput").ap(),
             2: nc.dram_tensor("res2", (B_CORE, D), bf16, kind="ExternalInput").ap()}
    # pre-transposed kv features: kvT8_m[p, rt, j, b] = kv[rt*128+b, j*128+p]
    kvT8_d, kvT16_d, w8_d, w16_d = {}, {}, {}, {}
    for m in (1, 2):
        if J:
            kvT8_d[m] = nc.dram_tensor(f"kvT8_{m}", (P, RT, J, P), f8,
                                       kind="ExternalInput").ap()
            w8_d[m] = nc.dram_tensor(f"w8_{m}", (P, J, D), f8,
                                     kind="ExternalInput").ap()
        if JB:
            kvT16_d[m] = nc.dram_tensor(f"kvT16_{m}", (P, RT, JB, P), bf16,
                                        kind="ExternalInput").ap()
            w16_d[m] = nc.dram_tensor(f"w16_{m}", (P, JB, D), bf16,
                                      kind="ExternalInput").ap()
    aux = {}
    for m, gb in ((1, gb1), (2, gb2)):
        if gb:
            aux[f"g{m}"] = nc.dram_tensor(f"g{m}", (1, D), f32,
                                          kind="ExternalInput").ap()
            aux[f"b{m}"] = nc.dram_tensor(f"b{m}", (1, D), f32,
                                          kind="ExternalInput").ap()
    out_d = {1: nc.dram_tensor("out1", (B_CORE, D), bf16, kind="ExternalOutput").ap(),
             2: nc.dram_tensor("out2", (B_CORE, D), bf16, kind="ExternalOutput").ap()}

    with tile.TileContext(nc) as tc:
        import contextlib
        with contextlib.ExitStack() as ctx:
            const = ctx.enter_context(tc.tile_pool(name="const", bufs=1))
            kvp8 = ctx.enter_context(tc.tile_pool(name="kvp8", bufs=4))
            kvp16 = ctx.enter_context(tc.tile_pool(name="kvp16", bufs=4))
            resp = ctx.enter_context(tc.tile_pool(name="resp", bufs=4))
            sp = ctx.enter_context(tc.tile_pool(name="sp", bufs=4))
            op = ctx.enter_context(tc.tile_pool(name="op", bufs=4))
            stat = ctx.enter_context(tc.tile_pool(name="stat", bufs=8))
            psum_o = ctx.enter_context(
                tc.tile_pool(name="psum_o", bufs=4, space="PSUM"))

            eps = const.tile([P, 1], f32, tag="eps")
            nc.vector.memset(eps, LN_EPS)
            if ADD_ENGINE == "pe":
                ident = const.tile([P, P], bf16, tag="ident")
                make_identity(nc, ident)

            # ---- DMA: batched x4-row-tile loads spread over 4 queues ----
            # sync: kvT8 + w8; vector: kvT16 + w16; gpsimd: res; scalar
            # issues nothing here (it triggers the output stores).
            BQ = 4            # row tiles per load batch
            NB = RT // BQ     # batches
            w8_sb, w16_sb = {}, {}

            def _load_w8(m, jp):
                if m not in w8_sb:
                    w8_sb[m] = const.tile([P, J, D], f8, tag=f"w8_{m}", name=f"w8_{m}")
                nc.sync.dma_start(w8_sb[m][:, 2 * jp:2 * jp + 2, :],
                                  w8_d[m][:, 2 * jp:2 * jp + 2, :])

            def _load_w16(m):
                wt = const.tile([P, JB, D], bf16, tag=f"w16_{m}", name=f"w16_{m}")
                for jj in range(JB):
                    nc.scalar.dma_start(wt[:, jj, :], w16_d[m][:, jj, :])
                w16_sb[m] = wt

            batches = {}

            def _load_batch(b):
                rt0 = b * BQ
                t = {}
                for m in (1, 2):
                    t8 = t16 = None
                    if J:
                        t8 = kvp8.tile([P, BQ, J, P], f8, tag=f"kvT8_{m}", name=f"kvT8_{m}_{b}")
                        nc.sync.dma_start(t8, kvT8_d[m][:, rt0:rt0 + BQ, :, :])
                    if JB:
                        t16 = kvp16.tile([P, BQ, JB, P], bf16, tag=f"kvT16_{m}", name=f"kvT16_{m}_{b}")
                        nc.scalar.dma_start(
                            t16, kvT16_d[m][:, rt0:rt0 + BQ, :, :])
                    tr = resp.tile([P, BQ, D], bf16, tag=f"res_{m}", name=f"res_{m}_{b}")
                    src = bass.AP(tensor=res_d[m].tensor,
                                  offset=res_d[m].offset + rt0 * P * D,
                                  ap=[[D, P], [P * D, BQ], [1, D]])
                    nc.gpsimd.dma_start(tr, src)
                    t[m] = (t8, t16, tr)
                batches[b] = t

            # head order: get mod-1's first DR matmul runnable ASAP
            if J:
                _load_w8(1, 0)
            _load_batch(0)
            for jp in range(1, J // 2):
                _load_w8(1, jp)
            if JB:
                _load_w16(1)
            for jp in range(J // 2):
                _load_w8(2, jp)
            if JB:
                _load_w16(2)

            aux_sb = {}
            for n, ap in aux.items():
                t = const.tile([P, D], f32, tag=n)
                bcast = bass.AP(tensor=ap.tensor, offset=ap.offset,
                                ap=[[0, P], ap.ap[1]])
                nc.sync.dma_start(t, bcast)
                aux_sb[n] = t

            # ---------------- main loop ----------------
            for rt in range(RT):
                b, q = divmod(rt, BQ)
                if b not in batches:
                    _load_batch(b)
                bt = batches[b]
                tiles = {m: (bt[m][0][:, q] if bt[m][0] is not None else None,
                             bt[m][1][:, q] if bt[m][1] is not None else None,
                             bt[m][2][:, q]) for m in (1, 2)}
                rows = slice(rt * P, (rt + 1) * P)

                for m in (1, 2):
                    t8, t16, tr = tiles[m]
                    ps = [psum_o.tile([P, 512], f32, tag=f"ps{nh}",
                                      name=f"ps_{rt}_{m}_{nh}")
                          for nh in range(2)]
                    # fp8 DoubleRow pairs, then bf16 chunks
                    for jp in range(J // 2):
                        for nh in range(2):
                            nc.tensor.matmul(
                                ps[nh],
                                t8[:, 2 * jp:2 * jp + 2, :],
                                w8_sb[m][:, 2 * jp:2 * jp + 2,
                                         nh * 512:(nh + 1) * 512],
                                start=(jp == 0),
                                stop=(JB == 0 and ADD_ENGINE != "pe"
                                      and jp == J // 2 - 1),
                                perf_mode=DR)
                    for jj in range(JB):
                        for nh in range(2):
                            nc.tensor.matmul(
                                ps[nh],
                                t16[:, jj, :],
                                w16_sb[m][:, jj, nh * 512:(nh + 1) * 512],
                                start=(J == 0 and jj == 0),
                                stop=(ADD_ENGINE != "pe" and jj == JB - 1))
                    if ADD_ENGINE == "pe":
                        for nh in range(2):
                            nc.tensor.matmul(
                                ps[nh], ident,
                                tr[:, nh * 512:(nh + 1) * 512],
                                start=False, stop=True)

                    if ADD_ENGINE == "pe":
                        s_sb = None
                        src0, src1 = ps[0], ps[1]
                    else:
                        s_sb = sp.tile([P, D], bf16, tag="s")
                        eng = nc.gpsimd if ADD_ENGINE == "pool" else nc.vector
                        for nh in range(2):
                            ncol = slice(nh * 512, (nh + 1) * 512)
                            eng.tensor_add(out=s_sb[:, ncol], in0=ps[nh],
                                           in1=tr[:, ncol])
                        src0, src1 = s_sb[:, 0:512], s_sb[:, 512:1024]

                    stats = stat.tile([P, 2, 6], f32, tag="stats")
                    nc.vector.bn_stats(stats[:, 0, :], src0)
                    nc.vector.bn_stats(stats[:, 1, :], src1)
                    mv = stat.tile([P, 2], f32, tag="mv")
                    nc.vector.bn_aggr(mv, stats)
                    # mv[:,1] <- 1/sqrt(var + eps)
                    nc.scalar.activation(
                        out=mv[:, 1:2], in_=mv[:, 1:2],
                        func=mybir.ActivationFunctionType.Sqrt,
                        bias=eps, scale=1.0)
                    nc.vector.reciprocal(mv[:, 1:2], mv[:, 1:2])
                    # nb = -mu * rstd so ACT computes s*rstd + nb
                    nb = stat.tile([P, 1], f32, tag="nb")
                    nc.vector.tensor_scalar(
                        out=nb, in0=mv[:, 0:1],
                        scalar1=mv[:, 1:2], scalar2=-1.0,
                        op0=mybir.AluOpType.mult,
                        op1=mybir.AluOpType.mult)

                    o_sb = op.tile([P, D], bf16, tag="o")
                    if ADD_ENGINE == "pe":
                        for nh in range(2):
                            nc.scalar.activation(
                                out=o_sb[:, nh * 512:(nh + 1) * 512],
                                in_=ps[nh],
                                func=mybir.ActivationFunctionType.Identity,
                                bias=nb, scale=mv[:, 1:2])
                    else:
                        nc.scalar.activation(
                            out=o_sb, in_=s_sb,
                            func=mybir.ActivationFunctionType.Identity,
                            bias=nb, scale=mv[:, 1:2])
                    if (gb1 if m == 1 else gb2):
                        nc.vector.tensor_mul(out=o_sb, in0=o_sb,
                                             in1=aux_sb[f"g{m}"])
                        nc.vector.tensor_add(out=o_sb, in0=o_sb,
                                             in1=aux_sb[f"b{m}"])
                    nc.gpsimd.dma_start(out_d[m][rows, :], o_sb)

    nc.compile()
    return nc


def _fold(in_w, in_b, out_w, out_b):
    Dv = out_w.shape[0]
    Wv = in_w[2 * Dv:3 * Dv, :].astype(np.float64)
    bv = in_b[2 * Dv:3 * Dv].astype(np.float64)
    W = (out_w.astype(np.float64) @ Wv).astype(np.float32)
    c = (bv @ out_w.astype(np.float64).T + out_b.astype(np.float64)
         ).astype(np.float32)
    return W, c


def _prep_w(W):
    """W [n,k] -> w8 [P, J, D] fp8 and w16 [P, KO-J, D] bf16 with
    w*[p, j, n] = W[n, j*128+p]."""
    wt = np.ascontiguousarray(W.T.reshape(KO, P, D).transpose(1, 0, 2))
    w8 = np.ascontiguousarray(wt[:, :J]).astype(NP_F8) if J else None
    w16 = (np.ascontiguousarray(wt[:, J:]).astype(NP_BF16)
           if KO - J else None)
    return w8, w16


def _prep_kvT(shard):
    """shard [B_CORE, D] -> kvT8 [P, RT, J, P] fp8, kvT16 [P, RT, KO-J, P]
    bf16 with kvT[p, rt, j, b] = shard[rt*128+b, j*128+p]."""
    x = shard.reshape(RT, P, KO, P).transpose(3, 0, 2, 1)
    kv8 = np.ascontiguousarray(x[:, :, :J]).astype(NP_F8) if J else None
    kv16 = (np.ascontiguousarray(x[:, :, J:]).astype(NP_BF16)
            if KO - J else None)
    return kv8, kv16


def kernel(image_features, text_features,
           in_w1, in_b1, out_w1, out_b1,
           in_w2, in_b2, out_w2, out_b2,
           ln1_g, ln1_b, ln2_g, ln2_b):
    from concourse import bass_utils

    image_features = np.ascontiguousarray(image_features, dtype=np.float32)
    text_features = np.ascontiguousarray(text_features, dtype=np.float32)

    W1, c1 = _fold(np.asarray(in_w1), np.asarray(in_b1),
                   np.asarray(out_w1), np.asarray(out_b1))
    W2, c2 = _fold(np.asarray(in_w2), np.asarray(in_b2),
                   np.asarray(out_w2), np.asarray(out_b2))
    w8_1, w16_1 = _prep_w(W1)
    w8_2, w16_2 = _prep_w(W2)

    flags = (bool(np.any(np.asarray(ln1_g) != 1) or np.any(np.asarray(ln1_b))),
             bool(np.any(np.asarray(ln2_g) != 1) or np.any(np.asarray(ln2_b))))

    if flags not in _PROGRAM_CACHE:
        _PROGRAM_CACHE[flags] = _build_program(flags)
    nc = _PROGRAM_CACHE[flags]

    in_maps = []
    for c in range(N_CORES):
        rows = slice(c * B_CORE, (c + 1) * B_CORE)
        img_shard = image_features[rows]
        txt_shard = text_features[rows]
        # mod 1: kv = txt, residual = img (+c1); mod 2: kv = img, res = txt
        kv8_1, kv16_1 = _prep_kvT(txt_shard)
        kv8_2, kv16_2 = _prep_kvT(img_shard)
        m = {
            "res1": (img_shard + c1).astype(NP_BF16),
            "res2": (txt_shard + c2).astype(NP_BF16),
        }
        if J:
            m.update({"kvT8_1": kv8_1, "kvT8_2": kv8_2,
                      "w8_1": w8_1, "w8_2": w8_2})
        if KO - J:
            m.update({"kvT16_1": kv16_1, "kvT16_2": kv16_2,
                      "w16_1": w16_1, "w16_2": w16_2})
        if flags[0]:
            m["g1"] = np.asarray(ln1_g, np.float32).reshape(1, D)
            m["b1"] = np.asarray(ln1_b, np.float32).reshape(1, D)
        if flags[1]:
            m["g2"] = np.asarray(ln2_g, np.float32).reshape(1, D)
            m["b2"] = np.asarray(ln2_b, np.float32).reshape(1, D)
        in_maps.append(m)

    global _LAST_IN_MAPS
    _LAST_IN_MAPS = in_maps
    res = bass_utils.run_bass_kernel_spmd(nc, in_maps, list(range(N_CORES)))
    attended_image = np.concatenate(
        [np.asarray(res.results[c]["out1"]) for c in range(N_CORES)],
        axis=0).astype(np.float32)
    attended_text = np.concatenate(
        [np.asarray(res.results[c]["out2"]) for c in range(N_CORES)],
        axis=0).astype(np.float32)
    return attended_image, attended_text


# revision 11
# speedup vs baseline: 1.0629x; 1.0629x over previous
"""CrossModalAttention Trainium2 kernel.

Math: with seq_len=1 on both query and key/value sides, softmax over the
single key is exactly 1.0, so MHA(q_in, kv_in) == (kv_in @ Wv.T + bv) @ out_w.T + out_b.
Folding the two projections on the host (in float64):
    W = out_w @ Wv          c = bv @ out_w.T + out_b
gives   out_m = LayerNorm(kv @ W.T + c + residual) * g + b.

Device work per modality: one [B,1024]x[1024,1024] matmul + residual add +
LayerNorm.  Sharding: pure data parallel over the batch dim, 8 cores.

Performance scheme (vs the fp32r baseline):
  * contraction split: first J of 8 k-chunks run as fp8(e4m3) DoubleRow
    matmuls (2 k-chunks per pass), the rest as bf16 matmuls.  J trades
    accuracy (fp8 quantization) for PE time; measured rel-err at J=6 is
    ~1.8e-2 vs the 2e-2 gate.
  * both modalities' lhsT are pre-transposed on the host (no PE
    transposes / PSUM->SBUF casts on device).
  * residual add runs on the Pool engine (gpsimd), LN stats on DVE,
    LN apply on the scalar engine; c (projection bias) is folded into
    the residual on the host.
  * all feature/weight traffic is bf16/fp8; outputs are written bf16
    and upcast on the host.
"""

import numpy as np
import ml_dtypes

P = 128          # partitions
D = 1024         # hidden dim
KO = D // P      # 8 contraction chunks
N_CORES = 8
B_FULL = 16384
B_CORE = B_FULL // N_CORES   # 2048
RT = B_CORE // P             # 16 row tiles per core
LN_EPS = 1e-5

J = 6            # fp8 k-chunks (DoubleRow pairs = J//2); 8-J chunks stay bf16
ADD_ENGINE = "pe"   # residual add: "pool" | "dve" | "pe"

NP_F8 = ml_dtypes.float8_e4m3
NP_BF16 = ml_dtypes.bfloat16

_PROGRAM_CACHE = {}


def _build_program(flags):
    """flags = (gb1, gb2): whether LN gamma/beta are non-trivial."""
    import concourse.bass as bass
    import concourse.bacc as bacc
    import concourse.tile as tile
    from concourse import mybir
    from concourse.masks import make_identity
    from concourse._compat import get_trn_type

    gb1, gb2 = flags
    f32 = mybir.dt.float32
    f8 = mybir.dt.float8e4
    bf16 = mybir.dt.bfloat16
    DR = mybir.MatmulPerfMode.DoubleRow
    JB = KO - J       # bf16 chunks

    nc = bacc.Bacc(get_trn_type() or "TRN2", target_bir_lowering=False,
                   debug=False, num_devices=N_CORES)

    # residuals (c folded in on host), [row, n] layout
    res_d = {1: nc.dram_tensor("res1", (B_CORE, D), bf16, kind="ExternalInput").ap(),
             2: nc.dram_tensor("res2", (B_CORE, D), bf16, kind="ExternalInput").ap()}
    # pre-transposed kv features: kvT8_m[p, rt, j, b] = kv[rt*128+b, j*128+p]
    kvT8_d, kvT16_d, w8_d, w16_d = {}, {}, {}, {}
    for m in (1, 2):
        if J:
            kvT8_d[m] = nc.dram_tensor(f"kvT8_{m}", (P, RT, J, P), f8,
                                       kind="ExternalInput").ap()
            w8_d[m] = nc.dram_tensor(f"w8_{m}", (P, J, D), f8,
                                     kind="ExternalInput").ap()
        if JB:
            kvT16_d[m] = nc.dram_tensor(f"kvT16_{m}", (P, RT, JB, P), bf16,
                                        kind="ExternalInput").ap()
            w16_d[m] = nc.dram_tensor(f"w16_{m}", (P, JB, D), bf16,
                                      kind="ExternalInput").ap()
    aux = {}
    for m, gb in ((1, gb1), (2, gb2)):
        if gb:
            aux[f"g{m}"] = nc.dram_tensor(f"g{m}", (1, D), f32,
                                          kind="ExternalInput").ap()
            aux[f"b{m}"] = nc.dram_tensor(f"b{m}", (1, D), f32,
                                          kind="ExternalInput").ap()
    out_d = {1: nc.dram_tensor("out1", (B_CORE, D), bf16, kind="ExternalOutput").ap(),
             2: nc.dram_tensor("out2", (B_CORE, D), bf16, kind="ExternalOutput").ap()}

    with tile.TileContext(nc) as tc:
        import contextlib
        with contextlib.ExitStack() as ctx:
            const = ctx.enter_context(tc.tile_pool(name="const", bufs=1))
            kvp8 = ctx.enter_context(tc.tile_pool(name="kvp8", bufs=4))
            kvp16 = ctx.enter_context(tc.tile_pool(name="kvp16", bufs=4))
            resp = ctx.enter_context(tc.tile_pool(name="resp", bufs=4))
            sp = ctx.enter_context(tc.tile_pool(name="sp", bufs=4))
            op = ctx.enter_context(tc.tile_pool(name="op", bufs=4))
            stat = ctx.enter_context(tc.tile_pool(name="stat", bufs=8))
            psum_o = ctx.enter_context(
                tc.tile_pool(name="psum_o", bufs=4, space="PSUM"))

            eps = const.tile([P, 1], f32, tag="eps")
            nc.vector.memset(eps, LN_EPS)
            if ADD_ENGINE == "pe":
                ident = const.tile([P, P], bf16, tag="ident")
                make_identity(nc, ident)

            # ---- DMA: batched x4-row-tile loads spread over 4 queues ----
            # sync: kvT8 + w8; vector: kvT16 + w16; gpsimd: res; scalar
            # issues nothing here (it triggers the output stores).
            BQ = 4            # row tiles per load batch
            NB = RT // BQ     # batches
            w8_sb, w16_sb = {}, {}

            def _load_w8(m, jp):
                if m not in w8_sb:
                    w8_sb[m] = const.tile([P, J, D], f8, tag=f"w8_{m}", name=f"w8_{m}")
                nc.sync.dma_start(w8_sb[m][:, 2 * jp:2 * jp + 2, :],
                                  w8_d[m][:, 2 * jp:2 * jp + 2, :])

            def _load_w16(m):
                wt = const.tile([P, JB, D], bf16, tag=f"w16_{m}", name=f"w16_{m}")
                for jj in range(JB):
                    nc.sync.dma_start(wt[:, jj, :], w16_d[m][:, jj, :])
                w16_sb[m] = wt

            batches = {}

            def _load_batch(b):
                rt0 = b * BQ
                t = {}
                for m in (1, 2):
                    t8 = t16 = None
                    if J:
                        t8 = kvp8.tile([P, BQ, J, P], f8, tag=f"kvT8_{m}", name=f"kvT8_{m}_{b}")
                        nc.sync.dma_start(t8, kvT8_d[m][:, rt0:rt0 + BQ, :, :])
                    if JB:
                        t16 = kvp16.tile([P, BQ, JB, P], bf16, tag=f"kvT16_{m}", name=f"kvT16_{m}_{b}")
                        nc.sync.dma_start(
                            t16, kvT16_d[m][:, rt0:rt0 + BQ, :, :])
                    trs = []
                    for q in range(BQ):
                        tr = resp.tile([P, D], bf16, tag=f"res_{m}",
                                       name=f"res_{m}_{rt0 + q}")
                        nc.gpsimd.dma_start(
                            tr, res_d[m][(rt0 + q) * P:(rt0 + q + 1) * P, :])
                        trs.append(tr)
                    t[m] = (t8, t16, trs)
                batches[b] = t

            # head order: get mod-1's first DR matmul runnable ASAP
            if J:
                _load_w8(1, 0)
            _load_batch(0)
            for jp in range(1, J // 2):
                _load_w8(1, jp)
            if JB:
                _load_w16(1)
            for jp in range(J // 2):
                _load_w8(2, jp)
            if JB:
                _load_w16(2)

            aux_sb = {}
            for n, ap in aux.items():
                t = const.tile([P, D], f32, tag=n)
                bcast = bass.AP(tensor=ap.tensor, offset=ap.offset,
                                ap=[[0, P], ap.ap[1]])
                nc.sync.dma_start(t, bcast)
                aux_sb[n] = t

            # ---------------- main loop ----------------
            for rt in range(RT):
                b, q = divmod(rt, BQ)
                if b not in batches:
                    _load_batch(b)
                bt = batches[b]
                tiles = {m: (bt[m][0][:, q] if bt[m][0] is not None else None,
                             bt[m][1][:, q] if bt[m][1] is not None else None,
                             bt[m][2][q]) for m in (1, 2)}
                rows = slice(rt * P, (rt + 1) * P)

                for m in (1, 2):
                    t8, t16, tr = tiles[m]
                    ps = [psum_o.tile([P, 512], f32, tag=f"ps{nh}",
                                      name=f"ps_{rt}_{m}_{nh}")
                          for nh in range(2)]
                    # fp8 DoubleRow pairs, then bf16 chunks
                    for jp in range(J // 2):
                        for nh in range(2):
                            nc.tensor.matmul(
                                ps[nh],
                                t8[:, 2 * jp:2 * jp + 2, :],
                                w8_sb[m][:, 2 * jp:2 * jp + 2,
                                         nh * 512:(nh + 1) * 512],
                                start=(jp == 0),
                                stop=(JB == 0 and ADD_ENGINE != "pe"
                                      and jp == J // 2 - 1),
                                perf_mode=DR)
                    for jj in range(JB):
                        for nh in range(2):
                            nc.tensor.matmul(
                                ps[nh],
                                t16[:, jj, :],
                                w16_sb[m][:, jj, nh * 512:(nh + 1) * 512],
                                start=(J == 0 and jj == 0),
                                stop=(ADD_ENGINE != "pe" and jj == JB - 1))
                    if ADD_ENGINE == "pe":
                        for nh in range(2):
                            nc.tensor.matmul(
                                ps[nh], ident,
                                tr[:, nh * 512:(nh + 1) * 512],
                                start=False, stop=True)

                    if ADD_ENGINE == "pe":
                        s_sb = None
                        src0, src1 = ps[0], ps[1]
                    else:
                        s_sb = sp.tile([P, D], bf16, tag="s")
                        eng = nc.gpsimd if ADD_ENGINE == "pool" else nc.vector
                        for nh in range(2):
                            ncol = slice(nh * 512, (nh + 1) * 512)
                            eng.tensor_add(out=s_sb[:, ncol], in0=ps[nh],
                                           in1=tr[:, ncol])
                        src0, src1 = s_sb[:, 0:512], s_sb[:, 512:1024]

                    stats = stat.tile([P, 2, 6], f32, tag="stats")
                    nc.vector.bn_stats(stats[:, 0, :], src0)
                    nc.vector.bn_stats(stats[:, 1, :], src1)
                    mv = stat.tile([P, 2], f32, tag="mv")
                    nc.vector.bn_aggr(mv, stats)
                    # mv[:,1] <- 1/sqrt(var + eps)
                    nc.scalar.activation(
                        out=mv[:, 1:2], in_=mv[:, 1:2],
                        func=mybir.ActivationFunctionType.Sqrt,
                        bias=eps, scale=1.0)
                    nc.vector.reciprocal(mv[:, 1:2], mv[:, 1:2])
                    # nb = -mu * rstd so ACT computes s*rstd + nb
                    nb = stat.tile([P, 1], f32, tag="nb")
                    nc.vector.tensor_scalar(
                        out=nb, in0=mv[:, 0:1],
                        scalar1=mv[:, 1:2], scalar2=-1.0,
                        op0=mybir.AluOpType.mult,
                        op1=mybir.AluOpType.mult)

                    o_sb = op.tile([P, D], bf16, tag="o")
                    if ADD_ENGINE == "pe":
                        for nh in range(2):
                            nc.scalar.activation(
                                out=o_sb[:, nh * 512:(nh + 1) * 512],
                                in_=ps[nh],
                                func=mybir.ActivationFunctionType.Identity,
                                bias=nb, scale=mv[:, 1:2])
                    else:
                        nc.scalar.activation(
                            out=o_sb, in_=s_sb,
                            func=mybir.ActivationFunctionType.Identity,
                            bias=nb, scale=mv[:, 1:2])
                    if (gb1 if m == 1 else gb2):
                        nc.vector.tensor_mul(out=o_sb, in0=o_sb,
                                             in1=aux_sb[f"g{m}"])
                        nc.vector.tensor_add(out=o_sb, in0=o_sb,
                                             in1=aux_sb[f"b{m}"])
                    nc.gpsimd.dma_start(out_d[m][rows, :], o_sb)

    nc.compile()
    return nc


def _fold(in_w, in_b, out_w, out_b):
    Dv = out_w.shape[0]
    Wv = in_w[2 * Dv:3 * Dv, :].astype(np.float64)
    bv = in_b[2 * Dv:3 * Dv].astype(np.float64)
    W = (out_w.astype(np.float64) @ Wv).astype(np.float32)
    c = (bv @ out_w.astype(np.float64).T + out_b.astype(np.float64)
         ).astype(np.float32)
    return W, c


def _prep_w(W):
    """W [n,k] -> w8 [P, J, D] fp8 and w16 [P, KO-J, D] bf16 with
    w*[p, j, n] = W[n, j*128+p]."""
    wt = np.ascontiguousarray(W.T.reshape(KO, P, D).transpose(1, 0, 2))
    w8 = np.ascontiguousarray(wt[:, :J]).astype(NP_F8) if J else None
    w16 = (np.ascontiguousarray(wt[:, J:]).astype(NP_BF16)
           if KO - J else None)
    return w8, w16


def _prep_kvT(shard):
    """shard [B_CORE, D] -> kvT8 [P, RT, J, P] fp8, kvT16 [P, RT, KO-J, P]
    bf16 with kvT[p, rt, j, b] = shard[rt*128+b, j*128+p]."""
    x = shard.reshape(RT, P, KO, P).transpose(3, 0, 2, 1)
    kv8 = np.ascontiguousarray(x[:, :, :J]).astype(NP_F8) if J else None
    kv16 = (np.ascontiguousarray(x[:, :, J:]).astype(NP_BF16)
            if KO - J else None)
    return kv8, kv16


def kernel(image_features, text_features,
           in_w1, in_b1, out_w1, out_b1,
           in_w2, in_b2, out_w2, out_b2,
           ln1_g, ln1_b, ln2_g, ln2_b):
    from concourse import bass_utils

    image_features = np.ascontiguousarray(image_features, dtype=np.float32)
    text_features = np.ascontiguousarray(text_features, dtype=np.float32)

    W1, c1 = _fold(np.asarray(in_w1), np.asarray(in_b1),
                   np.asarray(out_w1), np.asarray(out_b1))
    W2, c2 = _fold(np.asarray(in_w2), np.asarray(in_b2),
                   np.asarray(out_w2), np.asarray(out_b2))
    w8_1, w16_1 = _prep_w(W1)
    w8_2, w16_2 = _prep_w(W2)

    flags = (bool(np.any(np.asarray(ln1_g) != 1) or np.any(np.asarray(ln1_b))),
             bool(np.any(np.asarray(ln2_g) != 1) or np.any(np.asarray(ln2_b))))

    if flags not in _PROGRAM_CACHE:
        _PROGRAM_CACHE[flags] = _build_program(flags)
    nc = _PROGRAM_CACHE[flags]

    in_maps = []
    for c in range(N_CORES):
        rows = slice(c * B_CORE, (c + 1) * B_CORE)
        img_shard = image_features[rows]
        txt_shard = text_features[rows]
        # mod 1: kv = txt, residual = img (+c1); mod 2: kv = img, res = txt
        kv8_1, kv16_1 = _prep_kvT(txt_shard)
        kv8_2, kv16_2 = _prep_kvT(img_shard)
        m = {
            "res1": (img_shard + c1).astype(NP_BF16),
            "res2": (txt_shard + c2).astype(NP_BF16),
        }
        if J:
            m.update({"kvT8_1": kv8_1, "kvT8_2": kv8_2,
                      "w8_1": w8_1, "w8_2": w8_2})
        if KO - J:
            m.update({"kvT16_1": kv16_1, "kvT16_2": kv16_2,
                      "w16_1": w16_1, "w16_2": w16_2})
        if flags[0]:
            m["g1"] = np.asarray(ln1_g, np.float32).reshape(1, D)
            m["b1"] = np.asarray(ln1_b, np.float32).reshape(1, D)
        if flags[1]:
            m["g2"] = np.asarray(ln2_g, np.float32).reshape(1, D)
            m["b2"] = np.asarray(ln2_b, np.float32).reshape(1, D)
        in_maps.append(m)

    global _LAST_IN_MAPS
    _LAST_IN_MAPS = in_maps
    res = bass_utils.run_bass_kernel_spmd(nc, in_maps, list(range(N_CORES)))
    attended_image = np.concatenate(
        [np.asarray(res.results[c]["out1"]) for c in range(N_CORES)],
        axis=0).astype(np.float32)
    attended_text = np.concatenate(
        [np.asarray(res.results[c]["out2"]) for c in range(N_CORES)],
        axis=0).astype(np.float32)
    return attended_image, attended_text
